# revision 27
# baseline (speedup 1.0000x reference)
"""AnchorTargetLayer on 8 TRN2 NeuronCores.

Sharding: data-parallel over images N=4 x 2 K-halves (8 cores). Each core
computes, for its 120064-anchor slab of one image, the IoU assignment
(max/argmax over gt boxes), ambiguity flags, and the exact top-256
pos/neg subsample candidates (gpsimd topk). The host merges half slabs,
exactly re-resolves the rare numerically-ambiguous anchors, encodes
regression targets for positive anchors, and applies the subsample mask.

v2 layout: anchors are spatially sorted on the host and grouped into
cells of 134; each cell carries its own pruned gt list (conservative
bbox test, so excluded gts have IoU exactly 0). Cells are bin-packed
into (partition, slot) positions so that slot trip counts follow the
sorted cell-density quantiles (CAPS). The device loops gt-slot j with
per-partition scalars - every partition processes its own gt - via a
fused relu(min-max) custom DVE op, a fast reciprocal, and slab
reductions. Falls back to a dense 50-gt kernel if CAPS overflow.
"""

import sys
import numpy as np

for _p in ("/root/problem", "/opt/trn_rl_repo"):
    if _p not in sys.path:
        sys.path.insert(0, _p)

N, M, K = 4, 50, 240000
P, C = 128, 938          # device slab: 128 x 938 = 120064 anchors
KH = P * C
HALF1 = K - KH           # 119936
POS_T = np.float32(0.7)
NEG_T = np.float32(0.3)
NEG_INF = np.float32(-1e9)
EPS = np.float32(1e-8)
IDX_EPS = np.float32(2.0 ** -22)
GPAD = 64
DOCTOR = np.float32(-1e30)

COLS = 134               # cell width; 7 * 134 = 938
NSLOT = 7
CAPS = (31, 8, 6, 5, 4, 3, 3)   # per-slot gt trip counts (exact fit; dense fallback guards)
TOT = sum(CAPS)
NCELL = P * NSLOT
TOKV = 30080            # topk sub-vocab unit; token = TOKV (8tok) or 2*TOKV (4tok)
TOPK8 = False

_CACHE = {}


def _register_ovlr():
    """Custom DVE op: fused relu(min(Src0,C0) - max(Src1,C1))."""
    from concourse.dve_ops import OPS, DveOp, get_dve_sub_opcode
    from concourse.dve_spec import Spec, Src0, Src1, C0, C1, lower, minn, maxx, relu
    from concourse.dve_spec import _has_src1 as has_src1
    from concourse.dve_uop import DveOpSpec
    import concourse.dve_ops as D

    for op in OPS:
        if op.name == "ANT_OVLR":
            return op
    spec = Spec(
        body=relu(minn(Src0, C0) - maxx(Src1, C1)),
        reference=lambda in0, in1, s0, s1, imm2: np.maximum(
            np.minimum(in0, s0) - np.maximum(in1, s1), 0.0
        ).astype(np.float32),
    )
    op = DveOp("ANT_OVLR", spec, subdim=False, uops_sha={})
    OPS.append(op)
    D.CUSTOM_DVE_SPECS[op.name] = op.spec
    D._SUB_OPCODE_FOR_NAME[op.name] = D._CUSTOM_DVE_ROW_BASE + len(OPS) - 1
    assert max(D._SUB_OPCODE_FOR_NAME.values()) < 0x20
    for ver in ("v3", "v4"):
        sp = DveOpSpec(name=op.name, opcode=get_dve_sub_opcode(op.name),
                       uops=lower(spec, ver=ver), rd1_en=has_src1(spec))
        op.uops_sha[ver] = sp.sha(ver)
    return op


def _ovlr(nc, out, hi0, lo1, s_hi, s_lo):
    """out = relu(min(hi0, s_hi) - max(lo1, s_lo))"""
    op = _register_ovlr()
    return nc.vector._custom_dve(op, out=out, in0=hi0, in1=lo1, s0=s_hi, s1=s_lo)


def _register_iou1():
    """iou = inter * recip1(T - inter): seed + one Newton pass (~0.4% rel)."""
    from concourse.dve_ops import OPS, DveOp, get_dve_sub_opcode
    from concourse.dve_spec import Spec, Src0, Src1, C0, C1, lower, Bin, AluOp
    from concourse.dve_spec import _has_src1 as has_src1
    from concourse.dve_uop import DveOpSpec
    import concourse.dve_ops as D

    for op in OPS:
        if op.name == "ANT_IOU1":
            return op
    d = Src0 - Src1
    y0 = Bin(AluOp.BITWISE_NOT, d, d) * C0
    y1 = (C1 - d * y0) * y0

    def _ref(in0, in1, c0, c1, c2):
        dd = in0 - in1
        nx = (~dd.view(np.int32)).view(np.float32)
        yy0 = nx * c0
        yy1 = (c1 - dd * yy0) * yy0
        return (in1 * yy1).astype(np.float32)

    spec = Spec(body=Src1 * y1, reference=_ref)
    op = DveOp("ANT_IOU1", spec, subdim=False, uops_sha={})
    OPS.append(op)
    D.CUSTOM_DVE_SPECS[op.name] = op.spec
    D._SUB_OPCODE_FOR_NAME[op.name] = D._CUSTOM_DVE_ROW_BASE + len(OPS) - 1
    assert max(D._SUB_OPCODE_FOR_NAME.values()) < 0x20
    for ver in ("v3", "v4"):
        sp = DveOpSpec(name=op.name, opcode=get_dve_sub_opcode(op.name),
                       uops=lower(spec, ver=ver), rd1_en=has_src1(spec))
        op.uops_sha[ver] = sp.sha(ver)
    return op


def _iou1(nc, out, t_slab, inter):
    from concourse.dve_ops import RECIP_APPROX_FAST_CONSTS as RC
    op = _register_iou1()
    return nc.vector._custom_dve(op, out=out, in0=t_slab, in1=inter,
                                 s0=RC["s0"], s1=RC["s1"])



def _build_graph_v2():
    import concourse.mybir as mybir
    from concourse import bacc, tile

    f32 = mybir.dt.float32
    u32 = mybir.dt.uint32
    Op = mybir.AluOpType
    AX = mybir.AxisListType

    nc = bacc.Bacc()

    pin = {
        name: nc.declare_dram_parameter(name, [P, C], f32, isOutput=False)
        for name in ("ax1", "ay1", "ax2", "ay2", "score")
    }
    p_gt = nc.declare_dram_parameter("gt6", [P, TOT * 6], f32, isOutput=False)
    pout = {
        name: nc.declare_dram_parameter(name, [P, C], f32, isOutput=True)
        for name in ("maxo", "amd", "cnt")
    }
    NTOKp = 8 if TOPK8 else 4
    p_tk = nc.declare_dram_parameter("tk", [16 * NTOKp, 32], u32, isOutput=True)
    kb = nc.dram_tensor("kb", [2 * TOKV * 4], f32)

    NTOK = 8 if TOPK8 else 4
    with tile.TileContext(nc) as tc:
        t_tki = nc.alloc_sbuf_tensor("tki", [16 * NTOK, 8 * TOKV // NTOK // 16],
                                     f32, side="left")
        t_tko = nc.alloc_sbuf_tensor("tko", [16 * NTOK, 32], u32, side="left")

        with (
            tc.tile_pool(name="cons", bufs=1) as cp,
            tc.tile_pool(name="work", bufs=2) as wp,
            tc.tile_pool(name="jw", bufs=4) as jp,
        ):
            tin = {}
            for name in ("ax1", "ay1", "ax2", "ay2", "score"):
                t = cp.tile([P, C], f32, tag=name)
                nc.sync.dma_start(t[:], pin[name][:])
                tin[name] = t
            t_gt = cp.tile([P, TOT * 6], f32, tag="gt6")
            nc.sync.dma_start(t_gt[:], p_gt[:])

            stt = nc.vector.scalar_tensor_tensor



            t_aw = cp.tile([P, C], f32, tag="aw")
            t_ah = cp.tile([P, C], f32, tag="ah")
            t_area = cp.tile([P, C], f32, tag="area")
            stt(t_aw[:], tin["ax2"][:], 0.0, tin["ax1"][:], Op.bypass, Op.subtract)
            stt(t_ah[:], tin["ay2"][:], 0.0, tin["ay1"][:], Op.bypass, Op.subtract)
            stt(t_area[:], t_aw[:], 0.0, t_ah[:], Op.bypass, Op.mult)

            t_maxo = cp.tile([P, C], f32, tag="maxo")
            t_keym = cp.tile([P, C], f32, tag="keym")
            t_cnt = cp.tile([P, C], f32, tag="cnt")

            base = 0
            for s in range(NSLOT):
                cap = CAPS[s]
                R = slice(s * COLS, (s + 1) * COLS)
                slab_i = wp.tile([P, COLS, CAPS[0]], f32, tag="si")
                slab_r = wp.tile([P, COLS, CAPS[0]], f32, tag="sr")
                slab_t = wp.tile([P, COLS, CAPS[0]], f32, tag="st")
                for j in range(cap):
                    e = base + j
                    xw = jp.tile([P, COLS], f32, tag="xw")
                    yh = jp.tile([P, COLS], f32, tag="yh")
                    _ovlr(nc, xw[:], tin["ax2"][:, R], tin["ax1"][:, R],
                          t_gt[:, 2 * TOT + e:2 * TOT + e + 1],
                          t_gt[:, 0 * TOT + e:0 * TOT + e + 1])
                    _ovlr(nc, yh[:], tin["ay2"][:, R], tin["ay1"][:, R],
                          t_gt[:, 3 * TOT + e:3 * TOT + e + 1],
                          t_gt[:, 1 * TOT + e:1 * TOT + e + 1])
                    stt(slab_i[:, :, j], xw[:], 0.0, yh[:], Op.bypass, Op.mult)
                # T = area + areag ; iou = inter * recip1(T - inter)
                areag = t_gt[:, 4 * TOT + base:4 * TOT + base + cap]
                stt(slab_t[:, :, 0:cap],
                    t_area[:, R].unsqueeze(2).broadcast_to([P, COLS, cap]),
                    0.0, areag.unsqueeze(1).broadcast_to([P, COLS, cap]),
                    Op.bypass, Op.add)
                _iou1(nc, slab_i[:, :, 0:cap], slab_t[:, :, 0:cap],
                      slab_i[:, :, 0:cap])
                nc.vector.tensor_reduce(
                    t_maxo[:, R], slab_i[:, :, 0:cap], axis=AX.X, op=Op.max)
                # key = iou + idxw  (idxw strided per-entry scalar column)
                idxw = t_gt[:, 5 * TOT + base:5 * TOT + base + cap]
                nc.vector.scalar_tensor_tensor(
                    slab_r[:, :, 0:cap], slab_i[:, :, 0:cap], 0.0,
                    idxw.unsqueeze(1).broadcast_to([P, COLS, cap]),
                    Op.bypass, Op.add)
                nc.vector.tensor_reduce(
                    t_keym[:, R], slab_r[:, :, 0:cap], axis=AX.X, op=Op.max)
                # near-max count
                nc.vector.scalar_tensor_tensor(
                    slab_r[:, :, 0:cap],
                    t_maxo[:, R].unsqueeze(2).broadcast_to([P, COLS, cap]),
                    0.98, slab_i[:, :, 0:cap], Op.mult, Op.is_le)
                nc.vector.tensor_reduce(
                    t_cnt[:, R], slab_r[:, :, 0:cap], axis=AX.X, op=Op.add)
                # subsample keys for this column range, streamed to DRAM now
                t_m = jp.tile([P, COLS], f32, tag="km")
                t_u = jp.tile([P, COLS], f32, tag="ku")
                for cond_op, thr, which in ((Op.is_ge, 0.7, 0), (Op.is_lt, 0.3, 1)):
                    dst = jp.tile([P, COLS], f32, tag=f"kk{which}")
                    nc.vector.tensor_scalar(t_m[:], t_maxo[:, R],
                                            float(np.float32(thr)), None,
                                            op0=cond_op)
                    nc.vector.tensor_scalar(t_u[:], t_m[:], 1e9, -1e9,
                                            op0=Op.mult, op1=Op.add)
                    stt(dst[:], tin["score"][:, R], 0.0, t_m[:],
                        Op.bypass, Op.mult)
                    stt(dst[:], dst[:], 0.0, t_u[:], Op.bypass, Op.add)
                    kbv = kb[which * 4 * TOKV:
                             which * 4 * TOKV + KH].rearrange(
                                 "(p c) -> p c", c=C)[:, R]
                    nc.sync.dma_start(kbv, dst[:])
                base += cap

            t_amd = cp.tile([P, C], f32, tag="amd")
            stt(t_amd[:], t_keym[:], 0.0, t_maxo[:], Op.bypass, Op.subtract)

            # pad regions of both key blocks
            t_pad = cp.tile([1, 4 * TOKV - KH], f32, tag="pad")
            nc.vector.memset(t_pad[:], -1e30)
            nc.sync.dma_start(kb[KH:4 * TOKV].rearrange("(o f) -> o f", o=1),
                              t_pad[:])
            nc.sync.dma_start(kb[4 * TOKV + KH:8 * TOKV]
                              .rearrange("(o f) -> o f", o=1), t_pad[:])

            nc.sync.dma_start(
                t_tki[:], kb.rearrange("(t f) -> t f", f=8 * TOKV // NTOK // 16))
            if TOPK8:
                from concourse import bass_isa
                gp = nc.gpsimd
                gp.add_instruction(bass_isa.InstTopk(
                    name=f"I-{nc.next_id()}",
                    ins=[gp.lower_ap(t_tki[:], for_isa=True)],
                    outs=[gp.lower_ap(t_tko[:], for_isa=True)],
                    _tokens=8, _n=TOKV, _k=256,
                ))
            else:
                nc.gpsimd.topk(t_tko[:], t_tki[:], tokens=4,
                               vocab_size=2 * TOKV, k=256)
            # result/aux outputs stream while the topk scans
            nc.sync.dma_start(pout["maxo"][:], t_maxo[:])
            nc.sync.dma_start(pout["amd"][:], t_amd[:])
            nc.sync.dma_start(pout["cnt"][:], t_cnt[:])
            nc.sync.dma_start(p_tk[:, 0:32], t_tko[:])

    nc.compile()
    return nc


def _build_graph_dense():
    import concourse.mybir as mybir
    from concourse import bacc, tile

    f32 = mybir.dt.float32
    u32 = mybir.dt.uint32
    Op = mybir.AluOpType
    AX = mybir.AxisListType

    nc = bacc.Bacc()
    pin = {
        name: nc.declare_dram_parameter(name, [P, C], f32, isOutput=False)
        for name in ("ax1", "ay1", "ax2", "ay2", "score")
    }
    p_gt = nc.declare_dram_parameter("gtab", [P, 6 * GPAD], f32, isOutput=False)
    pout = {
        name: nc.declare_dram_parameter(name, [P, C], f32, isOutput=True)
        for name in ("maxo", "amd", "cnt")
    }
    p_tk = nc.declare_dram_parameter("tk", [64, 32], u32, isOutput=True)
    kb = nc.dram_tensor("kb", [2 * KH], f32)
    B = 16

    with tile.TileContext(nc) as tc:
        t_tki = nc.alloc_sbuf_tensor("tki", [64, 2 * KH // 64], f32, side="left")
        t_tko = nc.alloc_sbuf_tensor("tko", [64, 32], u32, side="left")
        with (
            tc.tile_pool(name="cons", bufs=1) as cp,
            tc.tile_pool(name="work", bufs=2) as wp,
        ):
            tin = {}
            for name in ("ax1", "ay1", "ax2", "ay2", "score"):
                t = cp.tile([P, C], f32, tag=name)
                nc.sync.dma_start(t[:], pin[name][:])
                tin[name] = t
            t_gt = cp.tile([P, 6 * GPAD], f32, tag="gt")
            nc.sync.dma_start(t_gt[:], p_gt[:])

            g_x1 = t_gt[:, 0:M]
            g_y1 = t_gt[:, GPAD:GPAD + M]
            g_x2 = t_gt[:, 2 * GPAD:2 * GPAD + M]
            g_y2 = t_gt[:, 3 * GPAD:3 * GPAD + M]
            g_areps = t_gt[:, 4 * GPAD:4 * GPAD + M]
            g_idxw = t_gt[:, 5 * GPAD:5 * GPAD + M]

            stt = nc.vector.scalar_tensor_tensor
            t_aw = cp.tile([P, C], f32, tag="aw")
            t_ah = cp.tile([P, C], f32, tag="ah")
            t_area = cp.tile([P, C], f32, tag="area")
            stt(t_aw[:], tin["ax2"][:], 0.0, tin["ax1"][:], Op.bypass, Op.subtract)
            stt(t_ah[:], tin["ay2"][:], 0.0, tin["ay1"][:], Op.bypass, Op.subtract)
            stt(t_area[:], t_aw[:], 0.0, t_ah[:], Op.bypass, Op.mult)

            t_maxo = cp.tile([P, C], f32, tag="maxo")
            t_keym = cp.tile([P, C], f32, tag="keym")
            t_cnt = cp.tile([P, C], f32, tag="cnt")

            def gbc(ap, Bs):
                return ap.unsqueeze(1).broadcast_to([P, Bs, M])

            def abc(ap, Bs):
                return ap.unsqueeze(2).broadcast_to([P, Bs, M])

            for s0 in range(0, C, B):
                Bs = min(B, C - s0)
                cols = slice(s0, s0 + Bs)
                sh = [P, Bs, M]
                t1 = wp.tile(sh, f32, tag="t1")
                t2 = wp.tile(sh, f32, tag="t2")
                wr = wp.tile(sh, f32, tag="wr")
                hr = wp.tile(sh, f32, tag="hr")
                inter = wp.tile(sh, f32, tag="inter")
                areaS = wp.tile(sh, f32, tag="areaS")
                rec = wp.tile(sh, f32, tag="rec")
                iou = wp.tile(sh, f32, tag="iou")
                key = wp.tile(sh, f32, tag="key")
                ge = wp.tile(sh, f32, tag="ge")
                stt(t1[:], gbc(g_x2, Bs), 0.0, abc(tin["ax2"][:, cols], Bs),
                    Op.bypass, Op.min)
                stt(t2[:], gbc(g_x1, Bs), 0.0, abc(tin["ax1"][:, cols], Bs),
                    Op.bypass, Op.max)
                stt(wr[:], t1[:], 0.0, t2[:], Op.bypass, Op.subtract)
                stt(t1[:], gbc(g_y2, Bs), 0.0, abc(tin["ay2"][:, cols], Bs),
                    Op.bypass, Op.min)
                stt(t2[:], gbc(g_y1, Bs), 0.0, abc(tin["ay1"][:, cols], Bs),
                    Op.bypass, Op.max)
                stt(hr[:], t1[:], 0.0, t2[:], Op.bypass, Op.subtract)
                stt(inter[:], wr[:], 0.0, hr[:], Op.max, Op.mult)
                stt(areaS[:], gbc(g_areps, Bs), 0.0, abc(t_area[:, cols], Bs),
                    Op.bypass, Op.add)
                stt(areaS[:], inter[:], -1.0, areaS[:], Op.mult, Op.add)
                nc.vector.reciprocal_approx_fast(rec[:], areaS[:])
                stt(iou[:], inter[:], 0.0, rec[:], Op.bypass, Op.mult)
                nc.vector.tensor_reduce(t_maxo[:, cols], iou[:], axis=AX.X, op=Op.max)
                stt(key[:], iou[:], 0.0, gbc(g_idxw, Bs), Op.bypass, Op.add)
                nc.vector.tensor_reduce(t_keym[:, cols], key[:], axis=AX.X, op=Op.max)
                stt(ge[:], abc(t_maxo[:, cols], Bs), 0.9999, iou[:],
                    Op.mult, Op.is_le)
                nc.vector.tensor_reduce(t_cnt[:, cols], ge[:], axis=AX.X, op=Op.add)

            t_amd = cp.tile([P, C], f32, tag="amd")
            stt(t_amd[:], t_keym[:], 0.0, t_maxo[:], Op.bypass, Op.subtract)
            nc.sync.dma_start(pout["maxo"][:], t_maxo[:])
            nc.sync.dma_start(pout["amd"][:], t_amd[:])
            nc.sync.dma_start(pout["cnt"][:], t_cnt[:])

            t_m = cp.tile([P, C], f32, tag="m")
            t_u = cp.tile([P, C], f32, tag="u")
            t_pk = cp.tile([P, C], f32, tag="pk")
            t_nk = cp.tile([P, C], f32, tag="nk")
            for cond_op, thr, dst in ((Op.is_ge, 0.7, t_pk), (Op.is_lt, 0.3, t_nk)):
                nc.vector.tensor_scalar(t_m[:], t_maxo[:], float(np.float32(thr)),
                                        None, op0=cond_op)
                nc.vector.tensor_scalar(t_u[:], t_m[:], 1e9, -1e9,
                                        op0=Op.mult, op1=Op.add)
                stt(dst[:], tin["score"][:], 0.0, t_m[:], Op.bypass, Op.mult)
                stt(dst[:], dst[:], 0.0, t_u[:], Op.bypass, Op.add)

            nc.sync.dma_start(kb[0:KH].rearrange("(p c) -> p c", c=C), t_pk[:])
            nc.sync.dma_start(kb[KH:2 * KH].rearrange("(p c) -> p c", c=C), t_nk[:])
            nc.sync.dma_start(t_tki[:], kb.rearrange("(t f) -> t f", f=2 * KH // 64))
            nc.gpsimd.topk(t_tko[:], t_tki[:], tokens=4, vocab_size=KH // 2, k=256)
            nc.sync.dma_start(p_tk[:], t_tko[:])

    nc.compile()
    return nc


def _get_nc(kind):
    key = "nc_" + kind
    if key not in _CACHE:
        _CACHE[key] = (_build_graph_v2 if kind == "v2" else _build_graph_dense)()
    return _CACHE[key]


# ---------------- host-side exact helpers ----------------

def _iou_rows(anchors_sub, g):
    a = anchors_sub.astype(np.float32)
    lt = np.maximum(a[:, None, :2], g[None, :, :2])
    rb = np.minimum(a[:, None, 2:], g[None, :, 2:])
    wh = np.maximum(rb - lt, np.float32(0.0))
    inter = wh[..., 0] * wh[..., 1]
    aa = (a[:, 2] - a[:, 0]) * (a[:, 3] - a[:, 1])
    ag = (g[:, 2] - g[:, 0]) * (g[:, 3] - g[:, 1])
    denom = aa[:, None] + ag[None, :] - inter + EPS
    return inter / denom


def _encode_rows(anchors_sub, matched):
    a = anchors_sub.astype(np.float32)
    m = matched.astype(np.float32)
    aw = a[:, 2] - a[:, 0]
    ah = a[:, 3] - a[:, 1]
    acx = a[:, 0] + np.float32(0.5) * aw
    acy = a[:, 1] + np.float32(0.5) * ah
    gw = m[:, 2] - m[:, 0]
    gh = m[:, 3] - m[:, 1]
    gcx = m[:, 0] + np.float32(0.5) * gw
    gcy = m[:, 1] + np.float32(0.5) * gh
    return np.stack([(gcx - acx) / aw, (gcy - acy) / ah,
                     np.log(gw / aw), np.log(gh / ah)], axis=-1)


def _topk_select(keyfull, vb_hint):
    valid = keyfull > NEG_INF / 2
    nvalid = int(valid.sum())
    if nvalid <= 256:
        return np.nonzero(valid)[0]
    vb = vb_hint
    gt_idx = np.nonzero(keyfull > vb)[0] if vb is not None else None
    if vb is None or len(gt_idx) > 256:
        vb = np.partition(keyfull, len(keyfull) - 256)[len(keyfull) - 256]
        gt_idx = np.nonzero(keyfull > vb)[0]
    eq_idx = np.nonzero(keyfull == vb)[0]
    if len(gt_idx) + len(eq_idx) < 256:
        vb = np.partition(keyfull, len(keyfull) - 256)[len(keyfull) - 256]
        gt_idx = np.nonzero(keyfull > vb)[0]
        eq_idx = np.nonzero(keyfull == vb)[0]
    need = 256 - len(gt_idx)
    sel = np.concatenate([gt_idx, eq_idx[:need]])
    return sel[keyfull[sel] > NEG_INF / 2]


def _spatial_perm(anchors):
    cx = (anchors[:, 0] + anchors[:, 2]) * 0.5
    cy = (anchors[:, 1] + anchors[:, 3]) * 0.5
    band = np.floor(cy / 24.0).astype(np.int64)
    return np.lexsort((cx, band))


def _pack_core(anchors, sc, g, overlap_mask, ids):
    """Build v2 per-core inputs. Returns (in_map, pos2id) or None on overflow."""
    cells_ids = ids.reshape(NCELL, COLS)
    ca = anchors[cells_ids]                      # [NCELL, COLS, 4]
    bx1 = ca[:, :, 0].min(1)
    by1 = ca[:, :, 1].min(1)
    bx2 = ca[:, :, 2].max(1)
    by2 = ca[:, :, 3].max(1)
    hit = ((g[None, :, 0] < bx2[:, None]) & (g[None, :, 2] > bx1[:, None]) &
           (g[None, :, 1] < by2[:, None]) & (g[None, :, 3] > by1[:, None]))
    Gc = hit.sum(1)
    order = np.argsort(-Gc, kind="stable")
    Gs = Gc[order]
    for s in range(NSLOT):
        if Gs[s * P] > CAPS[s]:
            return None
    pos2id = np.empty((P, C), np.int64)
    gt6 = np.zeros((6, TOT), np.float32)
    gt6[0] = 5000.0
    gt6[1] = 5000.0
    gt6[2] = 5001.0
    gt6[3] = 5001.0
    gt6[4] = np.float32(1.0) + EPS
    gt6[5] = -IDX_EPS
    gt6_all = np.broadcast_to(gt6[None], (P, 6, TOT)).copy()
    areag = ((g[:, 2] - g[:, 0]) * (g[:, 3] - g[:, 1]) + EPS).astype(np.float32)
    idxw = ((np.float32(M - 1) - np.arange(M, dtype=np.float32)) * IDX_EPS)
    base = 0
    for s in range(NSLOT):
        for p in range(P):
            cell = order[s * P + p]
            pos2id[p, s * COLS:(s + 1) * COLS] = cells_ids[cell]
            ms = np.nonzero(hit[cell])[0]
            ngt = len(ms)
            if ngt:
                gt6_all[p, 0:4, base:base + ngt] = g[ms].T
                gt6_all[p, 4, base:base + ngt] = areag[ms]
                gt6_all[p, 5, base:base + ngt] = idxw[ms]
        base += CAPS[s]
    sc_l = sc[pos2id].astype(np.float32)
    sc_l[overlap_mask[pos2id]] = DOCTOR
    in_map = {
        "ax1": anchors[pos2id, 0].astype(np.float32),
        "ay1": anchors[pos2id, 1].astype(np.float32),
        "ax2": anchors[pos2id, 2].astype(np.float32),
        "ay2": anchors[pos2id, 3].astype(np.float32),
        "score": sc_l,
        "gt6": gt6_all.reshape(P, TOT * 6),
    }
    return in_map, pos2id


def kernel(anchors, rpn_cls_score, gt_boxes, gt_labels):
    from concourse.bass_utils import run_bass_kernel_spmd

    anchors = np.ascontiguousarray(anchors, dtype=np.float32)
    scores = np.ascontiguousarray(rpn_cls_score, dtype=np.float32)
    gt_boxes = np.ascontiguousarray(gt_boxes, dtype=np.float32)
    gt_labels_np = np.ascontiguousarray(gt_labels)

    if "perm" not in _CACHE or _CACHE.get("perm_key") is not anchors:
        _CACHE["perm"] = _spatial_perm(anchors)
        _CACHE["perm_key"] = anchors
    perm = _CACHE["perm"]

    in_maps = []
    pos2ids = []
    use_v2 = True
    for core in range(8):
        n, h = core // 2, core % 2
        ids = perm[h * HALF1: h * HALF1 + KH]
        overlap_mask = np.zeros(K, bool)
        if h == 0:
            overlap_mask[perm[HALF1:KH]] = True
        r = _pack_core(anchors, scores[n], gt_boxes[n], overlap_mask, ids)
        if r is None:
            use_v2 = False
            break
        in_maps.append(r[0])
        pos2ids.append(r[1])

    if not use_v2:
        in_maps, pos2ids = [], []
        for core in range(8):
            n, h = core // 2, core % 2
            ids = perm[h * HALF1: h * HALF1 + KH]
            pos2id = ids.reshape(P, C)
            asl = anchors[pos2id]
            sc = scores[n][pos2id].astype(np.float32)
            if h == 0:
                om = np.zeros(K, bool)
                om[perm[HALF1:KH]] = True
                sc[om[pos2id]] = DOCTOR
            g = gt_boxes[n]
            gtab = np.zeros((6, GPAD), np.float32)
            gtab[0, :M] = g[:, 0]
            gtab[1, :M] = g[:, 1]
            gtab[2, :M] = g[:, 2]
            gtab[3, :M] = g[:, 3]
            gtab[4, :M] = (g[:, 2] - g[:, 0]) * (g[:, 3] - g[:, 1]) + EPS
            gtab[5, :M] = (np.float32(M - 1)
                           - np.arange(M, dtype=np.float32)) * IDX_EPS
            in_maps.append({
                "ax1": asl[:, :, 0].astype(np.float32),
                "ay1": asl[:, :, 1].astype(np.float32),
                "ax2": asl[:, :, 2].astype(np.float32),
                "ay2": asl[:, :, 3].astype(np.float32),
                "score": sc,
                "gtab": np.broadcast_to(gtab.reshape(1, 6 * GPAD),
                                        (P, 6 * GPAD)).copy(),
            })
            pos2ids.append(pos2id)

    nc = _get_nc("v2" if use_v2 else "dense")
    res = run_bass_kernel_spmd(nc, in_maps, list(range(8)))
    _CACHE["last_res"] = res
    results = res.results if hasattr(res, "results") else res

    cls_targets = np.zeros((N, K), np.int32)
    reg_targets = np.zeros((N, K, 4), np.float32)
    cls_weights = np.zeros((N, K), np.float32)
    reg_weights = np.zeros((N, K), np.float32)

    for n in range(N):
        maxo = np.empty(K, np.float32)
        amd = np.empty(K, np.float32)
        cnt = np.empty(K, np.float32)
        tks = {}
        p2i = {}
        for h in (0, 1):
            core = 2 * n + h
            r = results[core]
            pid = pos2ids[core].ravel()
            maxo[pid] = np.asarray(r["maxo"]).ravel()
            amd[pid] = np.asarray(r["amd"]).ravel()
            cnt[pid] = np.asarray(r["cnt"]).ravel()
            tks[h] = np.asarray(r["tk"])
            p2i[h] = pos2ids[core]

        am = (np.float32(M - 1)
              - np.round(amd * np.float32(2.0 ** 22))).astype(np.int64)
        np.clip(am, 0, M - 1, out=am)
        pos = maxo >= POS_T
        neg = maxo < NEG_T

        flag = (np.abs(maxo - POS_T) < np.float32(8e-3))
        flag |= (np.abs(maxo - NEG_T) < np.float32(8e-3))
        flag |= (cnt > 1.5) & (maxo > np.float32(0.45))
        fidx = np.nonzero(flag)[0]
        if len(fidx):
            io = _iou_rows(anchors[fidx], gt_boxes[n])
            mo_e = io.max(-1)
            am_e = io.argmax(-1)
            maxo[fidx] = mo_e
            am[fidx] = am_e
            pos[fidx] = mo_e >= POS_T
            neg[fidx] = mo_e < NEG_T

        pidx = np.nonzero(pos)[0]
        matched = gt_boxes[n][am[pidx]]
        enc = _encode_rows(anchors[pidx], matched)
        reg_targets[n, pidx] = enc
        cls_targets[n, pidx] = gt_labels_np[n][am[pidx]]

        mask = np.zeros(K, np.float32)
        for ktype, cond in ((0, pos), (1, neg)):
            keyfull = np.where(cond, scores[n], NEG_INF).astype(np.float32)
            vals = []
            for h in (0, 1):
                ntok = tks[h].shape[0] // 16 // 2  # tokens per key type
                for part in range(ntok):
                    tt = ntok * ktype + part
                    v = tks[h][16 * tt:16 * tt + 16, 0:16].reshape(256).view(np.float32)
                    vals.append(v)
            vals = np.concatenate(vals)
            vb = np.sort(vals)[-256] if len(vals) >= 256 else None
            sel = _topk_select(keyfull, vb)
            mask[sel] = 1.0

        cls_weights[n] = (pos | neg).astype(np.float32) * mask
        reg_weights[n] = pos.astype(np.float32) * mask

    return cls_targets, reg_targets, cls_weights, reg_weights


# revision 28
# speedup vs baseline: 1.1862x; 1.1862x over previous
"""AnchorTargetLayer on 8 TRN2 NeuronCores.

Sharding: data-parallel over images N=4 x 2 K-halves (8 cores). Each core
computes, for its 120064-anchor slab of one image, the IoU assignment
(max/argmax over gt boxes), ambiguity flags, and the exact top-256
pos/neg subsample candidates (gpsimd topk). The host merges half slabs,
exactly re-resolves the rare numerically-ambiguous anchors, encodes
regression targets for positive anchors, and applies the subsample mask.

v2 layout: anchors are spatially sorted on the host and grouped into
cells of 134; each cell carries its own pruned gt list (conservative
bbox test, so excluded gts have IoU exactly 0). Cells are bin-packed
into (partition, slot) positions so that slot trip counts follow the
sorted cell-density quantiles (CAPS). The device loops gt-slot j with
per-partition scalars - every partition processes its own gt - via a
fused relu(min-max) custom DVE op, a fast reciprocal, and slab
reductions. Falls back to a dense 50-gt kernel if CAPS overflow.
"""

import sys
import numpy as np

for _p in ("/root/problem", "/opt/trn_rl_repo"):
    if _p not in sys.path:
        sys.path.insert(0, _p)

N, M, K = 4, 50, 240000
P, C = 128, 938          # device slab: 128 x 938 = 120064 anchors
KH = P * C
HALF1 = K - KH           # 119936
POS_T = np.float32(0.7)
NEG_T = np.float32(0.3)
NEG_INF = np.float32(-1e9)
EPS = np.float32(1e-8)
IDX_EPS = np.float32(2.0 ** -22)
GPAD = 64
DOCTOR = np.float32(-1e30)

COLS = 134               # cell width; 7 * 134 = 938
NSLOT = 7
CAPS = (31, 8, 6, 5, 4, 3, 3)   # per-slot gt trip counts (exact fit; dense fallback guards)
TOT = sum(CAPS)
NCELL = P * NSLOT
TOKV = 30080            # topk sub-vocab unit; token = TOKV (8tok) or 2*TOKV (4tok)
TOPK8 = False

_CACHE = {}


def _register_ovlr():
    """Custom DVE op: fused relu(min(Src0,C0) - max(Src1,C1))."""
    from concourse.dve_ops import OPS, DveOp, get_dve_sub_opcode
    from concourse.dve_spec import Spec, Src0, Src1, C0, C1, lower, minn, maxx, relu
    from concourse.dve_spec import _has_src1 as has_src1
    from concourse.dve_uop import DveOpSpec
    import concourse.dve_ops as D

    for op in OPS:
        if op.name == "ANT_OVLR":
            return op
    spec = Spec(
        body=relu(minn(Src0, C0) - maxx(Src1, C1)),
        reference=lambda in0, in1, s0, s1, imm2: np.maximum(
            np.minimum(in0, s0) - np.maximum(in1, s1), 0.0
        ).astype(np.float32),
    )
    op = DveOp("ANT_OVLR", spec, subdim=False, uops_sha={})
    OPS.append(op)
    D.CUSTOM_DVE_SPECS[op.name] = op.spec
    D._SUB_OPCODE_FOR_NAME[op.name] = D._CUSTOM_DVE_ROW_BASE + len(OPS) - 1
    assert max(D._SUB_OPCODE_FOR_NAME.values()) < 0x20
    for ver in ("v3", "v4"):
        sp = DveOpSpec(name=op.name, opcode=get_dve_sub_opcode(op.name),
                       uops=lower(spec, ver=ver), rd1_en=has_src1(spec))
        op.uops_sha[ver] = sp.sha(ver)
    return op


def _ovlr(nc, out, hi0, lo1, s_hi, s_lo):
    """out = relu(min(hi0, s_hi) - max(lo1, s_lo))"""
    op = _register_ovlr()
    return nc.vector._custom_dve(op, out=out, in0=hi0, in1=lo1, s0=s_hi, s1=s_lo)


def _register_iou1():
    """iou = inter * recip1(T - inter): seed + one Newton pass (~0.4% rel)."""
    from concourse.dve_ops import OPS, DveOp, get_dve_sub_opcode
    from concourse.dve_spec import Spec, Src0, Src1, C0, C1, lower, Bin, AluOp
    from concourse.dve_spec import _has_src1 as has_src1
    from concourse.dve_uop import DveOpSpec
    import concourse.dve_ops as D

    for op in OPS:
        if op.name == "ANT_IOU1":
            return op
    d = Src0 - Src1
    y0 = Bin(AluOp.BITWISE_NOT, d, d) * C0
    y1 = (C1 - d * y0) * y0

    def _ref(in0, in1, c0, c1, c2):
        dd = in0 - in1
        nx = (~dd.view(np.int32)).view(np.float32)
        yy0 = nx * c0
        yy1 = (c1 - dd * yy0) * yy0
        return (in1 * yy1).astype(np.float32)

    spec = Spec(body=Src1 * y1, reference=_ref)
    op = DveOp("ANT_IOU1", spec, subdim=False, uops_sha={})
    OPS.append(op)
    D.CUSTOM_DVE_SPECS[op.name] = op.spec
    D._SUB_OPCODE_FOR_NAME[op.name] = D._CUSTOM_DVE_ROW_BASE + len(OPS) - 1
    assert max(D._SUB_OPCODE_FOR_NAME.values()) < 0x20
    for ver in ("v3", "v4"):
        sp = DveOpSpec(name=op.name, opcode=get_dve_sub_opcode(op.name),
                       uops=lower(spec, ver=ver), rd1_en=has_src1(spec))
        op.uops_sha[ver] = sp.sha(ver)
    return op


def _iou1(nc, out, t_slab, inter):
    from concourse.dve_ops import RECIP_APPROX_FAST_CONSTS as RC
    op = _register_iou1()
    return nc.vector._custom_dve(op, out=out, in0=t_slab, in1=inter,
                                 s0=RC["s0"], s1=RC["s1"])



def _build_graph_v2():
    import concourse.mybir as mybir
    from concourse import bacc, tile

    f32 = mybir.dt.float32
    u32 = mybir.dt.uint32
    Op = mybir.AluOpType
    AX = mybir.AxisListType

    nc = bacc.Bacc()

    pin = {
        name: nc.declare_dram_parameter(name, [P, C], f32, isOutput=False)
        for name in ("ax1", "ay1", "ax2", "ay2", "score")
    }
    p_gt = nc.declare_dram_parameter("gt6", [P, TOT * 6], f32, isOutput=False)
    pout = {
        name: nc.declare_dram_parameter(name, [P, C], f32, isOutput=True)
        for name in ("maxo", "amd", "cnt")
    }
    NTOKp = 8 if TOPK8 else 4
    p_tk = nc.declare_dram_parameter("tk", [16 * NTOKp, 32], u32, isOutput=True)
    kb = nc.dram_tensor("kb", [2 * TOKV * 4], f32)

    NTOK = 8 if TOPK8 else 4
    with tile.TileContext(nc) as tc:
        t_tki = nc.alloc_sbuf_tensor("tki", [16 * NTOK, 8 * TOKV // NTOK // 16],
                                     f32, side="left")
        t_tko = nc.alloc_sbuf_tensor("tko", [16 * NTOK, 32], u32, side="left")

        with (
            tc.tile_pool(name="cons", bufs=1) as cp,
            tc.tile_pool(name="work", bufs=2) as wp,
            tc.tile_pool(name="jw", bufs=4) as jp,
        ):
            t_gt = cp.tile([P, TOT * 6], f32, tag="gt6")
            nc.sync.dma_start(t_gt[:], p_gt[:])
            tin = {}
            for name in ("ax2", "ax1", "ay2", "ay1", "score"):
                t = cp.tile([P, C], f32, tag=name)
                if name in ("ax2", "ax1"):
                    nc.sync.dma_start(t[:, 0:COLS], pin[name][:, 0:COLS])
                    nc.sync.dma_start(t[:, COLS:], pin[name][:, COLS:])
                else:
                    nc.sync.dma_start(t[:], pin[name][:])
                tin[name] = t

            stt = nc.vector.scalar_tensor_tensor



            t_aw = cp.tile([P, C], f32, tag="aw")
            t_ah = cp.tile([P, C], f32, tag="ah")
            t_area = cp.tile([P, C], f32, tag="area")
            stt(t_aw[:], tin["ax2"][:], 0.0, tin["ax1"][:], Op.bypass, Op.subtract)
            stt(t_ah[:], tin["ay2"][:], 0.0, tin["ay1"][:], Op.bypass, Op.subtract)
            stt(t_area[:], t_aw[:], 0.0, t_ah[:], Op.bypass, Op.mult)

            t_maxo = cp.tile([P, C], f32, tag="maxo")
            t_keym = cp.tile([P, C], f32, tag="keym")
            t_cnt = cp.tile([P, C], f32, tag="cnt")

            base = 0
            for s in range(NSLOT):
                cap = CAPS[s]
                R = slice(s * COLS, (s + 1) * COLS)
                slab_i = wp.tile([P, COLS, CAPS[0]], f32, tag="si")
                slab_r = wp.tile([P, COLS, CAPS[0]], f32, tag="sr")
                slab_t = wp.tile([P, COLS, CAPS[0]], f32, tag="st")
                for j in range(cap):
                    e = base + j
                    xw = jp.tile([P, COLS], f32, tag="xw")
                    yh = jp.tile([P, COLS], f32, tag="yh")
                    _ovlr(nc, xw[:], tin["ax2"][:, R], tin["ax1"][:, R],
                          t_gt[:, 2 * TOT + e:2 * TOT + e + 1],
                          t_gt[:, 0 * TOT + e:0 * TOT + e + 1])
                    _ovlr(nc, yh[:], tin["ay2"][:, R], tin["ay1"][:, R],
                          t_gt[:, 3 * TOT + e:3 * TOT + e + 1],
                          t_gt[:, 1 * TOT + e:1 * TOT + e + 1])
                    stt(slab_i[:, :, j], xw[:], 0.0, yh[:], Op.bypass, Op.mult)
                # T = area + areag ; iou = inter * recip1(T - inter)
                areag = t_gt[:, 4 * TOT + base:4 * TOT + base + cap]
                stt(slab_t[:, :, 0:cap],
                    t_area[:, R].unsqueeze(2).broadcast_to([P, COLS, cap]),
                    0.0, areag.unsqueeze(1).broadcast_to([P, COLS, cap]),
                    Op.bypass, Op.add)
                _iou1(nc, slab_i[:, :, 0:cap], slab_t[:, :, 0:cap],
                      slab_i[:, :, 0:cap])
                nc.vector.tensor_reduce(
                    t_maxo[:, R], slab_i[:, :, 0:cap], axis=AX.X, op=Op.max)
                # key = iou + idxw  (idxw strided per-entry scalar column)
                idxw = t_gt[:, 5 * TOT + base:5 * TOT + base + cap]
                nc.vector.scalar_tensor_tensor(
                    slab_r[:, :, 0:cap], slab_i[:, :, 0:cap], 0.0,
                    idxw.unsqueeze(1).broadcast_to([P, COLS, cap]),
                    Op.bypass, Op.add)
                nc.vector.tensor_reduce(
                    t_keym[:, R], slab_r[:, :, 0:cap], axis=AX.X, op=Op.max)
                # near-max count
                nc.vector.scalar_tensor_tensor(
                    slab_r[:, :, 0:cap],
                    t_maxo[:, R].unsqueeze(2).broadcast_to([P, COLS, cap]),
                    0.98, slab_i[:, :, 0:cap], Op.mult, Op.is_le)
                nc.vector.tensor_reduce(
                    t_cnt[:, R], slab_r[:, :, 0:cap], axis=AX.X, op=Op.add)
                # subsample keys for this column range, streamed to DRAM now
                t_m = jp.tile([P, COLS], f32, tag="km")
                t_u = jp.tile([P, COLS], f32, tag="ku")
                for cond_op, thr, which in ((Op.is_ge, 0.7, 0), (Op.is_lt, 0.3, 1)):
                    dst = jp.tile([P, COLS], f32, tag=f"kk{which}")
                    nc.vector.tensor_scalar(t_m[:], t_maxo[:, R],
                                            float(np.float32(thr)), None,
                                            op0=cond_op)
                    nc.vector.tensor_scalar(t_u[:], t_m[:], 1e9, -1e9,
                                            op0=Op.mult, op1=Op.add)
                    stt(dst[:], tin["score"][:, R], 0.0, t_m[:],
                        Op.bypass, Op.mult)
                    stt(dst[:], dst[:], 0.0, t_u[:], Op.bypass, Op.add)
                    kbv = kb[which * 4 * TOKV:
                             which * 4 * TOKV + KH].rearrange(
                                 "(p c) -> p c", c=C)[:, R]
                    nc.sync.dma_start(kbv, dst[:])
                base += cap

            t_amd = cp.tile([P, C], f32, tag="amd")
            stt(t_amd[:], t_keym[:], 0.0, t_maxo[:], Op.bypass, Op.subtract)

            # pad regions of both key blocks
            t_pad = cp.tile([1, 4 * TOKV - KH], f32, tag="pad")
            nc.vector.memset(t_pad[:], -1e30)
            nc.sync.dma_start(kb[KH:4 * TOKV].rearrange("(o f) -> o f", o=1),
                              t_pad[:])
            nc.sync.dma_start(kb[4 * TOKV + KH:8 * TOKV]
                              .rearrange("(o f) -> o f", o=1), t_pad[:])

            nc.sync.dma_start(
                t_tki[:], kb.rearrange("(t f) -> t f", f=8 * TOKV // NTOK // 16))
            if TOPK8:
                from concourse import bass_isa
                gp = nc.gpsimd
                gp.add_instruction(bass_isa.InstTopk(
                    name=f"I-{nc.next_id()}",
                    ins=[gp.lower_ap(t_tki[:], for_isa=True)],
                    outs=[gp.lower_ap(t_tko[:], for_isa=True)],
                    _tokens=8, _n=TOKV, _k=256,
                ))
            else:
                nc.gpsimd.topk(t_tko[:], t_tki[:], tokens=4,
                               vocab_size=2 * TOKV, k=256)
            # result/aux outputs stream while the topk scans
            nc.sync.dma_start(pout["maxo"][:], t_maxo[:])
            nc.sync.dma_start(pout["amd"][:], t_amd[:])
            nc.sync.dma_start(pout["cnt"][:], t_cnt[:])
            nc.sync.dma_start(p_tk[:, 0:32], t_tko[:])

    nc.compile()
    return nc


def _build_graph_dense():
    import concourse.mybir as mybir
    from concourse import bacc, tile

    f32 = mybir.dt.float32
    u32 = mybir.dt.uint32
    Op = mybir.AluOpType
    AX = mybir.AxisListType

    nc = bacc.Bacc()
    pin = {
        name: nc.declare_dram_parameter(name, [P, C], f32, isOutput=False)
        for name in ("ax1", "ay1", "ax2", "ay2", "score")
    }
    p_gt = nc.declare_dram_parameter("gtab", [P, 6 * GPAD], f32, isOutput=False)
    pout = {
        name: nc.declare_dram_parameter(name, [P, C], f32, isOutput=True)
        for name in ("maxo", "amd", "cnt")
    }
    p_tk = nc.declare_dram_parameter("tk", [64, 32], u32, isOutput=True)
    kb = nc.dram_tensor("kb", [2 * KH], f32)
    B = 16

    with tile.TileContext(nc) as tc:
        t_tki = nc.alloc_sbuf_tensor("tki", [64, 2 * KH // 64], f32, side="left")
        t_tko = nc.alloc_sbuf_tensor("tko", [64, 32], u32, side="left")
        with (
            tc.tile_pool(name="cons", bufs=1) as cp,
            tc.tile_pool(name="work", bufs=2) as wp,
        ):
            tin = {}
            for name in ("ax1", "ay1", "ax2", "ay2", "score"):
                t = cp.tile([P, C], f32, tag=name)
                nc.sync.dma_start(t[:], pin[name][:])
                tin[name] = t
            t_gt = cp.tile([P, 6 * GPAD], f32, tag="gt")
            nc.sync.dma_start(t_gt[:], p_gt[:])

            g_x1 = t_gt[:, 0:M]
            g_y1 = t_gt[:, GPAD:GPAD + M]
            g_x2 = t_gt[:, 2 * GPAD:2 * GPAD + M]
            g_y2 = t_gt[:, 3 * GPAD:3 * GPAD + M]
            g_areps = t_gt[:, 4 * GPAD:4 * GPAD + M]
            g_idxw = t_gt[:, 5 * GPAD:5 * GPAD + M]

            stt = nc.vector.scalar_tensor_tensor
            t_aw = cp.tile([P, C], f32, tag="aw")
            t_ah = cp.tile([P, C], f32, tag="ah")
            t_area = cp.tile([P, C], f32, tag="area")
            stt(t_aw[:], tin["ax2"][:], 0.0, tin["ax1"][:], Op.bypass, Op.subtract)
            stt(t_ah[:], tin["ay2"][:], 0.0, tin["ay1"][:], Op.bypass, Op.subtract)
            stt(t_area[:], t_aw[:], 0.0, t_ah[:], Op.bypass, Op.mult)

            t_maxo = cp.tile([P, C], f32, tag="maxo")
            t_keym = cp.tile([P, C], f32, tag="keym")
            t_cnt = cp.tile([P, C], f32, tag="cnt")

            def gbc(ap, Bs):
                return ap.unsqueeze(1).broadcast_to([P, Bs, M])

            def abc(ap, Bs):
                return ap.unsqueeze(2).broadcast_to([P, Bs, M])

            for s0 in range(0, C, B):
                Bs = min(B, C - s0)
                cols = slice(s0, s0 + Bs)
                sh = [P, Bs, M]
                t1 = wp.tile(sh, f32, tag="t1")
                t2 = wp.tile(sh, f32, tag="t2")
                wr = wp.tile(sh, f32, tag="wr")
                hr = wp.tile(sh, f32, tag="hr")
                inter = wp.tile(sh, f32, tag="inter")
                areaS = wp.tile(sh, f32, tag="areaS")
                rec = wp.tile(sh, f32, tag="rec")
                iou = wp.tile(sh, f32, tag="iou")
                key = wp.tile(sh, f32, tag="key")
                ge = wp.tile(sh, f32, tag="ge")
                stt(t1[:], gbc(g_x2, Bs), 0.0, abc(tin["ax2"][:, cols], Bs),
                    Op.bypass, Op.min)
                stt(t2[:], gbc(g_x1, Bs), 0.0, abc(tin["ax1"][:, cols], Bs),
                    Op.bypass, Op.max)
                stt(wr[:], t1[:], 0.0, t2[:], Op.bypass, Op.subtract)
                stt(t1[:], gbc(g_y2, Bs), 0.0, abc(tin["ay2"][:, cols], Bs),
                    Op.bypass, Op.min)
                stt(t2[:], gbc(g_y1, Bs), 0.0, abc(tin["ay1"][:, cols], Bs),
                    Op.bypass, Op.max)
                stt(hr[:], t1[:], 0.0, t2[:], Op.bypass, Op.subtract)
                stt(inter[:], wr[:], 0.0, hr[:], Op.max, Op.mult)
                stt(areaS[:], gbc(g_areps, Bs), 0.0, abc(t_area[:, cols], Bs),
                    Op.bypass, Op.add)
                stt(areaS[:], inter[:], -1.0, areaS[:], Op.mult, Op.add)
                nc.vector.reciprocal_approx_fast(rec[:], areaS[:])
                stt(iou[:], inter[:], 0.0, rec[:], Op.bypass, Op.mult)
                nc.vector.tensor_reduce(t_maxo[:, cols], iou[:], axis=AX.X, op=Op.max)
                stt(key[:], iou[:], 0.0, gbc(g_idxw, Bs), Op.bypass, Op.add)
                nc.vector.tensor_reduce(t_keym[:, cols], key[:], axis=AX.X, op=Op.max)
                stt(ge[:], abc(t_maxo[:, cols], Bs), 0.9999, iou[:],
                    Op.mult, Op.is_le)
                nc.vector.tensor_reduce(t_cnt[:, cols], ge[:], axis=AX.X, op=Op.add)

            t_amd = cp.tile([P, C], f32, tag="amd")
            stt(t_amd[:], t_keym[:], 0.0, t_maxo[:], Op.bypass, Op.subtract)
            nc.sync.dma_start(pout["maxo"][:], t_maxo[:])
            nc.sync.dma_start(pout["amd"][:], t_amd[:])
            nc.sync.dma_start(pout["cnt"][:], t_cnt[:])

            t_m = cp.tile([P, C], f32, tag="m")
            t_u = cp.tile([P, C], f32, tag="u")
            t_pk = cp.tile([P, C], f32, tag="pk")
            t_nk = cp.tile([P, C], f32, tag="nk")
            for cond_op, thr, dst in ((Op.is_ge, 0.7, t_pk), (Op.is_lt, 0.3, t_nk)):
                nc.vector.tensor_scalar(t_m[:], t_maxo[:], float(np.float32(thr)),
                                        None, op0=cond_op)
                nc.vector.tensor_scalar(t_u[:], t_m[:], 1e9, -1e9,
                                        op0=Op.mult, op1=Op.add)
                stt(dst[:], tin["score"][:], 0.0, t_m[:], Op.bypass, Op.mult)
                stt(dst[:], dst[:], 0.0, t_u[:], Op.bypass, Op.add)

            nc.sync.dma_start(kb[0:KH].rearrange("(p c) -> p c", c=C), t_pk[:])
            nc.sync.dma_start(kb[KH:2 * KH].rearrange("(p c) -> p c", c=C), t_nk[:])
            nc.sync.dma_start(t_tki[:], kb.rearrange("(t f) -> t f", f=2 * KH // 64))
            nc.gpsimd.topk(t_tko[:], t_tki[:], tokens=4, vocab_size=KH // 2, k=256)
            nc.sync.dma_start(p_tk[:], t_tko[:])

    nc.compile()
    return nc


def _get_nc(kind):
    key = "nc_" + kind
    if key not in _CACHE:
        _CACHE[key] = (_build_graph_v2 if kind == "v2" else _build_graph_dense)()
    return _CACHE[key]


# ---------------- host-side exact helpers ----------------

def _iou_rows(anchors_sub, g):
    a = anchors_sub.astype(np.float32)
    lt = np.maximum(a[:, None, :2], g[None, :, :2])
    rb = np.minimum(a[:, None, 2:], g[None, :, 2:])
    wh = np.maximum(rb - lt, np.float32(0.0))
    inter = wh[..., 0] * wh[..., 1]
    aa = (a[:, 2] - a[:, 0]) * (a[:, 3] - a[:, 1])
    ag = (g[:, 2] - g[:, 0]) * (g[:, 3] - g[:, 1])
    denom = aa[:, None] + ag[None, :] - inter + EPS
    return inter / denom


def _encode_rows(anchors_sub, matched):
    a = anchors_sub.astype(np.float32)
    m = matched.astype(np.float32)
    aw = a[:, 2] - a[:, 0]
    ah = a[:, 3] - a[:, 1]
    acx = a[:, 0] + np.float32(0.5) * aw
    acy = a[:, 1] + np.float32(0.5) * ah
    gw = m[:, 2] - m[:, 0]
    gh = m[:, 3] - m[:, 1]
    gcx = m[:, 0] + np.float32(0.5) * gw
    gcy = m[:, 1] + np.float32(0.5) * gh
    return np.stack([(gcx - acx) / aw, (gcy - acy) / ah,
                     np.log(gw / aw), np.log(gh / ah)], axis=-1)


def _topk_select(keyfull, vb_hint):
    valid = keyfull > NEG_INF / 2
    nvalid = int(valid.sum())
    if nvalid <= 256:
        return np.nonzero(valid)[0]
    vb = vb_hint
    gt_idx = np.nonzero(keyfull > vb)[0] if vb is not None else None
    if vb is None or len(gt_idx) > 256:
        vb = np.partition(keyfull, len(keyfull) - 256)[len(keyfull) - 256]
        gt_idx = np.nonzero(keyfull > vb)[0]
    eq_idx = np.nonzero(keyfull == vb)[0]
    if len(gt_idx) + len(eq_idx) < 256:
        vb = np.partition(keyfull, len(keyfull) - 256)[len(keyfull) - 256]
        gt_idx = np.nonzero(keyfull > vb)[0]
        eq_idx = np.nonzero(keyfull == vb)[0]
    need = 256 - len(gt_idx)
    sel = np.concatenate([gt_idx, eq_idx[:need]])
    return sel[keyfull[sel] > NEG_INF / 2]


def _spatial_perm(anchors):
    cx = (anchors[:, 0] + anchors[:, 2]) * 0.5
    cy = (anchors[:, 1] + anchors[:, 3]) * 0.5
    band = np.floor(cy / 24.0).astype(np.int64)
    return np.lexsort((cx, band))


def _pack_core(anchors, sc, g, overlap_mask, ids):
    """Build v2 per-core inputs. Returns (in_map, pos2id) or None on overflow."""
    cells_ids = ids.reshape(NCELL, COLS)
    ca = anchors[cells_ids]                      # [NCELL, COLS, 4]
    bx1 = ca[:, :, 0].min(1)
    by1 = ca[:, :, 1].min(1)
    bx2 = ca[:, :, 2].max(1)
    by2 = ca[:, :, 3].max(1)
    hit = ((g[None, :, 0] < bx2[:, None]) & (g[None, :, 2] > bx1[:, None]) &
           (g[None, :, 1] < by2[:, None]) & (g[None, :, 3] > by1[:, None]))
    Gc = hit.sum(1)
    order = np.argsort(-Gc, kind="stable")
    Gs = Gc[order]
    for s in range(NSLOT):
        if Gs[s * P] > CAPS[s]:
            return None
    pos2id = np.empty((P, C), np.int64)
    gt6 = np.zeros((6, TOT), np.float32)
    gt6[0] = 5000.0
    gt6[1] = 5000.0
    gt6[2] = 5001.0
    gt6[3] = 5001.0
    gt6[4] = np.float32(1.0) + EPS
    gt6[5] = -IDX_EPS
    gt6_all = np.broadcast_to(gt6[None], (P, 6, TOT)).copy()
    areag = ((g[:, 2] - g[:, 0]) * (g[:, 3] - g[:, 1]) + EPS).astype(np.float32)
    idxw = ((np.float32(M - 1) - np.arange(M, dtype=np.float32)) * IDX_EPS)
    base = 0
    for s in range(NSLOT):
        for p in range(P):
            cell = order[s * P + p]
            pos2id[p, s * COLS:(s + 1) * COLS] = cells_ids[cell]
            ms = np.nonzero(hit[cell])[0]
            ngt = len(ms)
            if ngt:
                gt6_all[p, 0:4, base:base + ngt] = g[ms].T
                gt6_all[p, 4, base:base + ngt] = areag[ms]
                gt6_all[p, 5, base:base + ngt] = idxw[ms]
        base += CAPS[s]
    sc_l = sc[pos2id].astype(np.float32)
    sc_l[overlap_mask[pos2id]] = DOCTOR
    in_map = {
        "ax1": anchors[pos2id, 0].astype(np.float32),
        "ay1": anchors[pos2id, 1].astype(np.float32),
        "ax2": anchors[pos2id, 2].astype(np.float32),
        "ay2": anchors[pos2id, 3].astype(np.float32),
        "score": sc_l,
        "gt6": gt6_all.reshape(P, TOT * 6),
    }
    return in_map, pos2id


def kernel(anchors, rpn_cls_score, gt_boxes, gt_labels):
    from concourse.bass_utils import run_bass_kernel_spmd

    anchors = np.ascontiguousarray(anchors, dtype=np.float32)
    scores = np.ascontiguousarray(rpn_cls_score, dtype=np.float32)
    gt_boxes = np.ascontiguousarray(gt_boxes, dtype=np.float32)
    gt_labels_np = np.ascontiguousarray(gt_labels)

    if "perm" not in _CACHE or _CACHE.get("perm_key") is not anchors:
        _CACHE["perm"] = _spatial_perm(anchors)
        _CACHE["perm_key"] = anchors
    perm = _CACHE["perm"]

    in_maps = []
    pos2ids = []
    use_v2 = True
    for core in range(8):
        n, h = core // 2, core % 2
        ids = perm[h * HALF1: h * HALF1 + KH]
        overlap_mask = np.zeros(K, bool)
        if h == 0:
            overlap_mask[perm[HALF1:KH]] = True
        r = _pack_core(anchors, scores[n], gt_boxes[n], overlap_mask, ids)
        if r is None:
            use_v2 = False
            break
        in_maps.append(r[0])
        pos2ids.append(r[1])

    if not use_v2:
        in_maps, pos2ids = [], []
        for core in range(8):
            n, h = core // 2, core % 2
            ids = perm[h * HALF1: h * HALF1 + KH]
            pos2id = ids.reshape(P, C)
            asl = anchors[pos2id]
            sc = scores[n][pos2id].astype(np.float32)
            if h == 0:
                om = np.zeros(K, bool)
                om[perm[HALF1:KH]] = True
                sc[om[pos2id]] = DOCTOR
            g = gt_boxes[n]
            gtab = np.zeros((6, GPAD), np.float32)
            gtab[0, :M] = g[:, 0]
            gtab[1, :M] = g[:, 1]
            gtab[2, :M] = g[:, 2]
            gtab[3, :M] = g[:, 3]
            gtab[4, :M] = (g[:, 2] - g[:, 0]) * (g[:, 3] - g[:, 1]) + EPS
            gtab[5, :M] = (np.float32(M - 1)
                           - np.arange(M, dtype=np.float32)) * IDX_EPS
            in_maps.append({
                "ax1": asl[:, :, 0].astype(np.float32),
                "ay1": asl[:, :, 1].astype(np.float32),
                "ax2": asl[:, :, 2].astype(np.float32),
                "ay2": asl[:, :, 3].astype(np.float32),
                "score": sc,
                "gtab": np.broadcast_to(gtab.reshape(1, 6 * GPAD),
                                        (P, 6 * GPAD)).copy(),
            })
            pos2ids.append(pos2id)

    nc = _get_nc("v2" if use_v2 else "dense")
    res = run_bass_kernel_spmd(nc, in_maps, list(range(8)))
    _CACHE["last_res"] = res
    results = res.results if hasattr(res, "results") else res

    cls_targets = np.zeros((N, K), np.int32)
    reg_targets = np.zeros((N, K, 4), np.float32)
    cls_weights = np.zeros((N, K), np.float32)
    reg_weights = np.zeros((N, K), np.float32)

    for n in range(N):
        maxo = np.empty(K, np.float32)
        amd = np.empty(K, np.float32)
        cnt = np.empty(K, np.float32)
        tks = {}
        p2i = {}
        for h in (0, 1):
            core = 2 * n + h
            r = results[core]
            pid = pos2ids[core].ravel()
            maxo[pid] = np.asarray(r["maxo"]).ravel()
            amd[pid] = np.asarray(r["amd"]).ravel()
            cnt[pid] = np.asarray(r["cnt"]).ravel()
            tks[h] = np.asarray(r["tk"])
            p2i[h] = pos2ids[core]

        am = (np.float32(M - 1)
              - np.round(amd * np.float32(2.0 ** 22))).astype(np.int64)
        np.clip(am, 0, M - 1, out=am)
        pos = maxo >= POS_T
        neg = maxo < NEG_T

        flag = (np.abs(maxo - POS_T) < np.float32(8e-3))
        flag |= (np.abs(maxo - NEG_T) < np.float32(8e-3))
        flag |= (cnt > 1.5) & (maxo > np.float32(0.45))
        fidx = np.nonzero(flag)[0]
        if len(fidx):
            io = _iou_rows(anchors[fidx], gt_boxes[n])
            mo_e = io.max(-1)
            am_e = io.argmax(-1)
            maxo[fidx] = mo_e
            am[fidx] = am_e
            pos[fidx] = mo_e >= POS_T
            neg[fidx] = mo_e < NEG_T

        pidx = np.nonzero(pos)[0]
        matched = gt_boxes[n][am[pidx]]
        enc = _encode_rows(anchors[pidx], matched)
        reg_targets[n, pidx] = enc
        cls_targets[n, pidx] = gt_labels_np[n][am[pidx]]

        mask = np.zeros(K, np.float32)
        for ktype, cond in ((0, pos), (1, neg)):
            keyfull = np.where(cond, scores[n], NEG_INF).astype(np.float32)
            vals = []
            for h in (0, 1):
                ntok = tks[h].shape[0] // 16 // 2  # tokens per key type
                for part in range(ntok):
                    tt = ntok * ktype + part
                    v = tks[h][16 * tt:16 * tt + 16, 0:16].reshape(256).view(np.float32)
                    vals.append(v)
            vals = np.concatenate(vals)
            vb = np.sort(vals)[-256] if len(vals) >= 256 else None
            sel = _topk_select(keyfull, vb)
            mask[sel] = 1.0

        cls_weights[n] = (pos | neg).astype(np.float32) * mask
        reg_weights[n] = pos.astype(np.float32) * mask

    return cls_targets, reg_targets, cls_weights, reg_weights
